# revision 16
# baseline (speedup 1.0000x reference)
"""Trainium2 Bass kernel for nn_Decoder (Bahdanau attention + GRU step + vocab projection).

Sharding: data-parallel over batch (32 batches/core) for attention+GRU;
tensor-parallel over vocab (4000 cols/core) for the output projection, with an
on-device AllGather of the (transposed) GRU state between the two phases.

Matmuls with free dim >= 256 run as float32r (tf32, full PE rate); small/precision-
critical matmuls (context, softmax reductions, transposes) run exact float32.

Per-batch pipeline fuses score -> tanh -> V-dot -> softmax -> context so the
encoder tensor streams through SBUF once per layout and stays under the PE shadow.

Outputs (matching the reference): (logits [256,32000], state [256,1024], attn [256,128,1]).
"""
import sys

sys.path.insert(0, "/opt/trn_rl_repo")

import numpy as np

# Model dims (hardcoded per the task contract)
VOCAB, EMBED, UNITS, B, T, ENC = 32000, 512, 1024, 256, 128, 1024
N_CORES = 8
BC = B // N_CORES          # 32 batches per core
VC = VOCAB // N_CORES      # 4000 vocab cols per core
GU = 2 * UNITS             # used GRU gate cols (z and h blocks; r is unused)
F = ENC + EMBED            # 1536 GRU input features

_CACHE = {}


def _round_tf32(a):
    """Round-to-nearest-even to tf32 (10-bit mantissa); zeroes low 13 mantissa bits."""
    u = np.ascontiguousarray(a, dtype=np.float32).view(np.uint32)
    u = (u + 0xFFF + ((u >> 13) & 1)) & np.uint32(0xFFFFE000)
    return u.view(np.float32)


def _build_nc(no_cc=False, stop_after=99):
    import concourse.mybir as mybir
    import concourse.tile as tile
    from concourse import bacc
    from concourse.bass import ds, _add_dep_helper

    f32 = mybir.dt.float32
    f32r = mybir.dt.float32r
    AF = mybir.ActivationFunctionType
    OP = mybir.AluOpType

    nc = bacc.Bacc("TRN2", target_bir_lowering=False, debug=False)

    # ---- DRAM I/O ----  (f32r tensors are pre-rounded to tf32 on the host)
    enc_d = nc.dram_tensor("enc", [BC, T, ENC], f32r, kind="ExternalInput")
    encT_d = nc.dram_tensor("encT", [BC, ENC, T], f32r, kind="ExternalInput")
    hT_d = nc.dram_tensor("hT", [UNITS, BC], f32r, kind="ExternalInput")
    eT_d = nc.dram_tensor("embT", [EMBED, BC], f32r, kind="ExternalInput")
    w1_d = nc.dram_tensor("W1", [ENC, UNITS], f32r, kind="ExternalInput")
    w2_d = nc.dram_tensor("W2", [UNITS, UNITS], f32r, kind="ExternalInput")
    v_d = nc.dram_tensor("V", [1, UNITS], f32, kind="ExternalInput")
    b12_d = nc.dram_tensor("b12", [1, UNITS], f32r, kind="ExternalInput")
    wx_d = nc.dram_tensor("Wx", [F, GU], f32r, kind="ExternalInput")
    bg_d = nc.dram_tensor("bg", [1, GU], f32r, kind="ExternalInput")
    wfc_d = nc.dram_tensor("Wfc", [UNITS, VC], f32r, kind="ExternalInput")
    bfc_d = nc.dram_tensor("bfc", [1, VC], f32r, kind="ExternalInput")
    ones_d = nc.dram_tensor("ones", [1, 128], f32r, kind="ExternalInput")

    logits_d = nc.dram_tensor("logits", [B, VC], f32, kind="ExternalOutput")
    state_d = nc.dram_tensor("state", [BC, UNITS], f32, kind="ExternalOutput")
    attn_d = nc.dram_tensor("attn", [BC, T], f32r, kind="ExternalOutput")

    q_dram = nc.dram_tensor("q_dram", [BC, UNITS], f32r)
    ctx_dram = nc.dram_tensor("ctx_dram", [BC, ENC], f32)
    sT_bounce = nc.dram_tensor("sT_bounce", [UNITS, BC], f32r)
    ag_shared = nc.dram_tensor("ag_shared", [N_CORES * UNITS, BC], f32r, addr_space="Shared")

    w1_re = w1_d[:].rearrange("(ko p) c -> p ko c", p=128)      # [128,8,1024]
    w2_re = w2_d[:].rearrange("(ko p) c -> p ko c", p=128)      # [128,8,1024]
    hT_re = hT_d[:].rearrange("(ko p) b -> p ko b", p=128)      # [128,8,32]
    eT_re = eT_d[:].rearrange("(ko p) b -> p ko b", p=128)      # [128,4,32]
    wx_re = wx_d[:].rearrange("(ko p) g -> p ko g", p=128)      # [128,12,2048]
    wfc_re = wfc_d[:].rearrange("(ko p) v -> p ko v", p=128)    # [128,8,4000]
    ag_view = ag_shared[:].rearrange("(c u) b -> u c b", c=N_CORES)  # [1024,8,32]

    # logits n-tiling over this core's 4000 vocab cols
    NSZ = [512] * 7 + [416]
    NOFF = [sum(NSZ[:i]) for i in range(len(NSZ))]

    with tile.TileContext(nc) as tc:
        with (
            tc.tile_pool(name="const", bufs=1) as constp,
            tc.tile_pool(name="mm_ps", bufs=4, space="PSUM") as mm_ps,
            tc.tile_pool(name="misc_ps", bufs=3, space="PSUM") as misc_ps,
            tc.tile_pool(name="work", bufs=2) as work,
            tc.tile_pool(name="small", bufs=1) as small,
            tc.tile_pool(name="biasp", bufs=2) as biasp,
            tc.tile_pool(name="crow", bufs=2) as crowp,
            tc.tile_pool(name="wxp", bufs=12) as wxp,
            tc.tile_pool(name="wfcp", bufs=18) as wfcp,
            tc.tile_pool(name="loutp", bufs=4) as loutp,
        ):
            # ---------- constants (W1/W2 split per k-tile so matmuls pipeline with DMA) ----------
            w1_t = []
            for k in range(8):
                t = constp.tile([128, UNITS], f32r, tag=f"w1_{k}")
                nc.sync.dma_start(out=t[:], in_=w1_re[:, k, :])
                w1_t.append(t)
            hT_sb = constp.tile([128, 8, BC], f32r, tag="hT")
            nc.sync.dma_start(out=hT_sb[:], in_=hT_re)
            eT_sb = constp.tile([128, 4, BC], f32r, tag="eT")
            nc.sync.dma_start(out=eT_sb[:], in_=eT_re)
            v_row = constp.tile([1, UNITS], f32, tag="vrow")
            nc.sync.dma_start(out=v_row[:], in_=v_d[:])
            b12_row = constp.tile([1, UNITS], f32r, tag="b12")
            nc.sync.dma_start(out=b12_row[:], in_=b12_d[:])

            ones_row = constp.tile([1, 128], f32, tag="ones_row")
            nc.vector.memset(ones_row[:], 1.0)
            ones_row_r = constp.tile([1, 128], f32r, tag="ones_row_r")
            nc.sync.dma_start(out=ones_row_r[:], in_=ones_d[:])
            ones_col = constp.tile([128, 1], f32, tag="ones_col")
            nc.vector.memset(ones_col[:], 1.0)
            ident1 = constp.tile([32, 32], f32, tag="id1")
            nc.gpsimd.memset(ident1[:], 0.0)
            nc.gpsimd.affine_select(
                out=ident1[:], in_=ident1[:],
                compare_op=OP.not_equal, fill=1.0,
                base=0, pattern=[[-1, 32]], channel_multiplier=1,
            )

            # V broadcast to all partitions: vbc[p, c] = V[c]  (exact fp32 matmul)
            v_bc = constp.tile([128, UNITS], f32, tag="vbc")
            for n in range(2):
                vps = misc_ps.tile([128, 512], f32, tag="misc")
                nc.tensor.matmul(vps[:], ones_row[:, 0:128], v_row[:, ds(n * 512, 512)],
                                 start=True, stop=True)
                nc.vector.tensor_copy(v_bc[:, ds(n * 512, 512)], vps[:])

            # ---------- phase 0: q = hidden @ W2 + (b1+b2), layout [b=32, u=1024] ----------
            # q rows are bounced through DRAM so each batch can load its row at partition 0.
            q_sb = constp.tile([BC, UNITS], f32r, tag="q")
            with tc.tile_pool(name="w2p", bufs=3) as w2p:
                for n in range(2):
                    qps = mm_ps.tile([BC, 512], f32, tag="mm")
                    nc.tensor.matmul(qps[:], ones_row_r[:, 0:BC],
                                     b12_row[:, ds(n * 512, 512)],
                                     start=True, stop=False)
                    for k in range(8):
                        w2t = w2p.tile([128, 512], f32r, tag="w2")
                        nc.sync.dma_start(out=w2t[:], in_=w2_re[:, k, ds(n * 512, 512)])
                        nc.tensor.matmul(qps[:], hT_sb[:, k, :], w2t[:],
                                         start=False, stop=(k == 7))
                    nc.vector.tensor_copy(q_sb[:, ds(n * 512, 512)], qps[:])
            q_store = nc.sync.dma_start(out=q_dram[:], in_=q_sb[:])

            # weight prefetch (SWDGE ring) is emitted inside the batch loop at b==2
            wx_tiles = {}
            wx_order = [(n, k) for n in range(4) for k in range(12)]
            wfc_tiles = {}
            wfc_order = [(n, k) for n in range(8) for k in range(8)]

            def emit_prefetch():
                for (n, k) in wx_order[:12]:
                    wt = wxp.tile([128, 512], f32r, tag="wx")
                    nc.gpsimd.dma_start(out=wt[:], in_=wx_re[:, k, ds(n * 512, 512)])
                    wx_tiles[(n, k)] = wt
                for (n, k) in wfc_order[:18]:
                    nsz_, noff_ = NSZ[n], NOFF[n]
                    wt = wfcp.tile([128, 512], f32r, tag="wfc")
                    nc.gpsimd.dma_start(out=wt[:, 0:nsz_],
                                        in_=wfc_re[:, k, ds(noff_, nsz_)])
                    wfc_tiles[(n, k)] = wt

            # ---------- phase 1: per-batch score -> tanh -> V-dot -> softmax -> context ----------
            s_tb = small.tile([T, BC], f32, tag="s_tb")
            attn_tb = small.tile([T, BC], f32r, tag="attn_tb")
            e_tb = small.tile([T, BC], f32, tag="e_tb")
            r1 = small.tile([1, BC], f32, tag="r1")
            if stop_after >= 1:
                with tc.tile_pool(name="encTp", bufs=2) as encTp, \
                     tc.tile_pool(name="encNp", bufs=3) as encNp, \
                     tc.tile_pool(name="qrow", bufs=2) as qrowp:
                    crow_dmas = []
                    for b in range(BC):
                        if b == 2:
                            emit_prefetch()
                        encT_t = encTp.tile([128, 8, T], f32r, tag="encT")
                        nc.sync.dma_start(
                            out=encT_t[:],
                            in_=encT_d[b].rearrange("(ko p) t -> p ko t", p=128))
                        enc_t = encNp.tile([T, ENC], f32r, tag="encN")
                        nc.sync.dma_start(out=enc_t[:], in_=enc_d[b])
                        q_row = qrowp.tile([1, UNITS], f32r, tag="qrow")
                        qd = nc.scalar.dma_start(out=q_row[:], in_=q_dram[ds(b, 1), :])
                        _add_dep_helper(qd.ins, q_store.ins, sync=True,
                                        reason="q row after q store")
                        ts_t = work.tile([T, UNITS], f32, tag="ts")
                        for n in range(2):
                            ps = mm_ps.tile([T, 512], f32, tag="mm")
                            # q broadcast: ps[t, c] = q[b, c]
                            nc.tensor.matmul(ps[:], ones_row_r[:, 0:T],
                                             q_row[:, ds(n * 512, 512)],
                                             start=True, stop=False)
                            for k in range(8):
                                nc.tensor.matmul(ps[:], encT_t[:, k, :],
                                                 w1_t[k][:, ds(n * 512, 512)],
                                                 start=False, stop=(k == 7))
                            nc.scalar.activation(ts_t[:, ds(n * 512, 512)], ps[:], AF.Tanh)
                        scr = work.tile([T, UNITS], f32, tag="scr")
                        nc.vector.scalar_tensor_tensor(
                            out=scr[:], in0=ts_t[:], scalar=1.0, in1=v_bc[:],
                            op0=OP.mult, op1=OP.mult, accum_out=s_tb[:, ds(b, 1)],
                        )
                        # per-batch softmax over t (partition dim): exp -> sum -> recip -> scale
                        nc.scalar.activation(e_tb[:, ds(b, 1)], s_tb[:, ds(b, 1)], AF.Exp)
                        sum_ps = misc_ps.tile([1, 1], f32, tag="misc")
                        nc.tensor.matmul(sum_ps[:], e_tb[:, ds(b, 1)], ones_col[:],
                                         start=True, stop=True)
                        nc.vector.reciprocal(r1[:, ds(b, 1)], sum_ps[:])
                        rbc_ps = misc_ps.tile([T, 1], f32, tag="misc")
                        nc.tensor.matmul(rbc_ps[:], ones_row[:, 0:T], r1[:, ds(b, 1)],
                                         start=True, stop=True)
                        nc.vector.tensor_tensor(out=attn_tb[:, ds(b, 1)],
                                                in0=e_tb[:, ds(b, 1)], in1=rbc_ps[:],
                                                op=OP.mult)
                        # context row b (exact fp32): ctx[b, e] = sum_t attn[t] enc[t, e]
                        crow_sb = crowp.tile([1, ENC], f32, tag="crow")
                        for h in range(2):
                            cps = misc_ps.tile([1, 512], f32, tag="misc")
                            nc.tensor.matmul(cps[:], attn_tb[:, ds(b, 1)],
                                             enc_t[:, ds(h * 512, 512)],
                                             start=True, stop=True)
                            nc.scalar.activation(crow_sb[:, ds(h * 512, 512)], cps[:],
                                                 AF.Copy)
                        cd = nc.scalar.dma_start(out=ctx_dram[ds(b, 1), :], in_=crow_sb[:])
                        crow_dmas.append(cd)
                nc.scalar.dma_start(out=attn_d[:].rearrange("b t -> t b"), in_=attn_tb[:])

            if stop_after >= 4:
                # ---------- phase 4: GRU gates gx = xt @ Wx_used + bg  -> state ----------
                ctx_rows = small.tile([BC, ENC], f32, tag="ctx_rows")
                cg = nc.sync.dma_start(out=ctx_rows[:], in_=ctx_dram[:])
                for cd in crow_dmas:
                    _add_dep_helper(cg.ins, cd.ins, sync=True, reason="gather after rows")
                cT_ps = misc_ps.tile([128, 8 * BC], f32, tag="misc")
                for ch in range(8):
                    nc.tensor.transpose(cT_ps[:, ds(ch * BC, BC)],
                                        ctx_rows[:, ds(ch * 128, 128)], ident1[:])
                ctxT_sb = small.tile([128, 8, BC], f32r, tag="ctxT")
                nc.vector.tensor_copy(ctxT_sb[:],
                                      cT_ps[:].rearrange("p (c b) -> p c b", b=BC))

                def xt_tile(k):
                    if k < 8:
                        return ctxT_sb[:, k, :]
                    return eT_sb[:, k - 8, :]

                t1 = small.tile([BC, UNITS], f32, tag="t1")
                t2 = small.tile([BC, UNITS], f32, tag="t2")
                for n in range(4):
                    gps = mm_ps.tile([BC, 512], f32, tag="mm")
                    bg_row = biasp.tile([1, 512], f32r, tag="bias")
                    nc.sync.dma_start(out=bg_row[:], in_=bg_d[:, ds(n * 512, 512)])
                    nc.tensor.matmul(gps[:], ones_row_r[:, 0:BC], bg_row[:],
                                     start=True, stop=False)
                    for k in range(12):
                        if (n, k) in wx_tiles:
                            wx_t = wx_tiles.pop((n, k))
                        else:
                            wx_t = wxp.tile([128, 512], f32r, tag="wx")
                            nc.gpsimd.dma_start(out=wx_t[:],
                                                in_=wx_re[:, k, ds(n * 512, 512)])
                        nc.tensor.matmul(gps[:], xt_tile(k), wx_t[:],
                                         start=False, stop=(k == 11))
                    if n < 2:  # z block: t1 = tanh(0.5*gz)  (sigmoid via tanh)
                        nc.scalar.activation(t1[:, ds(n * 512, 512)], gps[:],
                                             AF.Tanh, scale=0.5)
                    else:      # h block: t2 = tanh(gh)
                        nc.scalar.activation(t2[:, ds((n - 2) * 512, 512)], gps[:], AF.Tanh)

                # state = (1-z)*hh = 0.5*(t2 - t1*t2)
                t3 = work.tile([BC, UNITS], f32, tag="ts")
                nc.vector.tensor_tensor(out=t3[:], in0=t1[:], in1=t2[:], op=OP.mult)
                t4 = work.tile([BC, UNITS], f32, tag="scr")
                nc.vector.tensor_tensor(out=t4[:], in0=t2[:], in1=t3[:], op=OP.subtract)
                state_sb = work.tile([BC, UNITS], f32, tag="ts")
                nc.vector.tensor_scalar_mul(state_sb[:], t4[:], 0.5)
                nc.scalar.dma_start(out=state_d[:], in_=state_sb[:])

                # stateT = 0.5 * t4^T  via PE transpose, scaling in the PSUM->SBUF copy
                sT_ps = misc_ps.tile([128, 8 * BC], f32, tag="misc")
                for ch in range(8):
                    nc.tensor.transpose(sT_ps[:, ds(ch * BC, BC)],
                                        t4[:, ds(ch * 128, 128)], ident1[:])
                sT_sb = small.tile([128, 8, BC], f32r, tag="sT")
                nc.vector.tensor_scalar_mul(
                    sT_sb[:], sT_ps[:].rearrange("p (c b) -> p c b", b=BC), 0.5)

                # ---------- phase 5: AllGather state^T across cores ----------
                store = nc.sync.dma_start(
                    out=sT_bounce[:].rearrange("(c p) b -> p c b", p=128), in_=sT_sb[:])
                if not no_cc:
                    cc = nc.gpsimd.collective_compute(
                        "AllGather", OP.bypass,
                        replica_groups=[list(range(N_CORES))],
                        ins=[sT_bounce[:].opt()],
                        outs=[ag_shared[:].opt()],
                    )
                    _add_dep_helper(cc.ins, store.ins, sync=True,
                                    reason="collective after state store")

            if stop_after >= 5:
                # ---------- phase 6: logits = state @ Wfc + bfc over this core's vocab ----------
                wfcB = tc.tile_pool(name="wfcB", bufs=12)
                wfcBp = wfcB.__enter__()
                for (n, k) in wfc_order[18:18 + 12]:
                    nsz_, noff_ = NSZ[n], NOFF[n]
                    wt = wfcBp.tile([128, 512], f32r, tag="wfcB")
                    nc.gpsimd.dma_start(out=wt[:, 0:nsz_], in_=wfc_re[:, k, ds(noff_, nsz_)])
                    wfc_tiles[(n, k)] = wt
                sT_tiles = {}
                for k in range(8):
                    for m in range(2):
                        stt = constp.tile([128, 4, BC], f32r, tag=f"sTt{k}_{m}")
                        d = nc.sync.dma_start(
                            out=stt[:], in_=ag_view[ds(k * 128, 128), ds(m * 4, 4), :])
                        if not no_cc:
                            _add_dep_helper(d.ins, cc.ins, sync=True,
                                            reason="read after allgather")
                        sT_tiles[(k, m)] = stt

                for n in range(8):
                    nsz, noff = NSZ[n], NOFF[n]
                    wfc_ts = []
                    for k in range(8):
                        if (n, k) in wfc_tiles:
                            wt = wfc_tiles.pop((n, k))
                        else:
                            wt = wfcp.tile([128, 512], f32r, tag="wfc")
                            nc.gpsimd.dma_start(out=wt[:, 0:nsz],
                                                in_=wfc_re[:, k, ds(noff, nsz)])
                        wfc_ts.append(wt)
                    bfc_row = biasp.tile([1, 512], f32r, tag="bias")
                    nc.sync.dma_start(out=bfc_row[:, 0:nsz], in_=bfc_d[:, ds(noff, nsz)])
                    for m in range(2):
                        lps = mm_ps.tile([128, 512], f32, tag="mm")
                        nc.tensor.matmul(lps[:, 0:nsz], ones_row_r[:, 0:128],
                                         bfc_row[:, 0:nsz],
                                         start=True, stop=False)
                        for k in range(8):
                            nc.tensor.matmul(
                                lps[:, 0:nsz],
                                sT_tiles[(k, m)][:].rearrange("p a b -> p (a b)"),
                                wfc_ts[k][:, 0:nsz],
                                start=False, stop=(k == 7))
                        lout = loutp.tile([128, 512], f32, tag="lout")
                        nc.vector.tensor_copy(lout[:, 0:nsz], lps[:, 0:nsz])
                        nc.scalar.dma_start(out=logits_d[ds(m * 128, 128), ds(noff, nsz)],
                                          in_=lout[:, 0:nsz])
            if stop_after >= 5:
                wfcB.__exit__(None, None, None)
    nc.finalize()
    return nc


def _prep_inputs(x, hidden, enc_output, E, W1, b1, W2, b2, V, bv, Wx, Wh, b_gru, Wfc, bfc):
    """Host-side sharding / layout prep. Returns per-core input maps."""
    x = np.asarray(x)
    hidden = np.asarray(hidden, dtype=np.float32)
    enc = np.ascontiguousarray(np.asarray(enc_output, dtype=np.float32))
    E = np.asarray(E, dtype=np.float32)
    W1 = _round_tf32(np.asarray(W1, dtype=np.float32))
    W2 = _round_tf32(np.asarray(W2, dtype=np.float32))
    V = np.asarray(V, dtype=np.float32).reshape(-1)
    Wx = np.asarray(Wx, dtype=np.float32)
    b_gru = np.asarray(b_gru, dtype=np.float32).reshape(-1)
    Wfc = _round_tf32(np.asarray(Wfc, dtype=np.float32))
    bfc = np.asarray(bfc, dtype=np.float32).reshape(-1)
    b12 = _round_tf32((np.asarray(b1, dtype=np.float32)
                       + np.asarray(b2, dtype=np.float32)).reshape(1, UNITS))

    encT = _round_tf32(enc.transpose(0, 2, 1))                    # [B, ENC, T]
    emb = E[np.asarray(x).reshape(-1)]                            # [B, EMBED]
    embT = _round_tf32(emb.T)                                     # [EMBED, B]
    hiddenT = _round_tf32(hidden.T)                               # [UNITS, B]
    Wx_used = _round_tf32(
        np.concatenate([Wx[:, :UNITS], Wx[:, 2 * UNITS:]], axis=1))  # [F, 2U]
    bg_used = _round_tf32(
        np.concatenate([b_gru[:UNITS], b_gru[2 * UNITS:]]).reshape(1, GU))
    v_row = V.reshape(1, UNITS)
    bfc_r = _round_tf32(bfc)
    in_maps = []
    for c in range(N_CORES):
        bs = slice(c * BC, (c + 1) * BC)
        vs = slice(c * VC, (c + 1) * VC)
        in_maps.append({
            "enc": np.ascontiguousarray(enc[bs]),
            "encT": np.ascontiguousarray(encT[bs]),
            "hT": np.ascontiguousarray(hiddenT[:, bs]),
            "embT": np.ascontiguousarray(embT[:, bs]),
            "W1": W1,
            "W2": W2,
            "V": v_row,
            "b12": b12,
            "Wx": Wx_used,
            "bg": bg_used,
            "Wfc": np.ascontiguousarray(Wfc[:, vs]),
            "bfc": np.ascontiguousarray(bfc_r[vs]).reshape(1, VC),
            "ones": np.ones((1, 128), np.float32),
        })
    return in_maps


def kernel(**inputs):
    from concourse.bass_utils import run_bass_kernel_spmd

    if "nc" not in _CACHE:
        _CACHE["nc"] = _build_nc()
    nc = _CACHE["nc"]

    in_maps = _prep_inputs(**inputs)
    res = run_bass_kernel_spmd(nc, in_maps, list(range(N_CORES)))
    outs = res.results

    logits = np.concatenate([outs[c]["logits"] for c in range(N_CORES)], axis=1)
    state = np.concatenate([outs[c]["state"] for c in range(N_CORES)], axis=0)
    attn = np.concatenate([outs[c]["attn"] for c in range(N_CORES)], axis=0)
    return (
        logits.astype(np.float32),
        state.astype(np.float32),
        attn.reshape(B, T, 1).astype(np.float32),
    )
